# revision 6
# baseline (speedup 1.0000x reference)
"""Expert-parallel sparse MoE kernel for Trainium2 (8 NeuronCores).

Reference model: dense MoE (every expert on every token) followed by a
top-2-sparse combine, residual add, and LayerNorm.  Mathematically only the
top-2 experts per token contribute to the output, so the kernel routes each
token to its top-2 experts and only computes those expert FFNs.

Sharding: expert-parallel.  Each of the 8 cores owns 8 of the 64 experts and
receives the tokens routed to them (all-to-all by routing, done host-side as
part of sharding).  The device streams the expert weights (the dominant
memory traffic, cast to bf16) and computes y_e = relu(x @ W1[e] + b1[e]) @
W2[e] for every routed token.  The host applies the gate weights + b2 during
the unshard/scatter, adds the residual, and normalizes.
"""

import numpy as np
import ml_dtypes

B, S, D, H, E, TOPK = 2, 1024, 512, 2048, 64, 2
T = B * S
NCORES = 8
EPC = E // NCORES          # experts per core
CAP = 128                  # token capacity per expert (observed max ~95)
DC = D // 128              # 4 contraction chunks for x @ W1
HC = H // 128              # 16 contraction chunks for h @ W2
EPS = 1e-5
BF16 = ml_dtypes.bfloat16

PROFILE = False            # set True (module-level) to capture an NTFF trace
LAST_RESULT = None         # BassKernelResults of the last run (for test.py)

_NC_CACHE = {}


def _build_bass():
    """Build the per-core Bass/Tile program (identical on all 8 cores)."""
    import concourse.bacc as bacc
    import concourse.mybir as mybir
    from concourse import tile

    nc = bacc.Bacc("TRN2", target_bir_lowering=False, debug=False,
                   num_devices=NCORES)

    bf = mybir.dt.bfloat16
    f32 = mybir.dt.float32
    xt = nc.dram_tensor("xt", [128, EPC, DC, CAP], bf, kind="ExternalInput")
    w1 = nc.dram_tensor("w1", [EPC, 128, DC, H], bf, kind="ExternalInput")
    w2 = nc.dram_tensor("w2", [EPC, 128, HC, D], bf, kind="ExternalInput")
    b1 = nc.dram_tensor("b1", [128, EPC, HC], f32, kind="ExternalInput")
    y = nc.dram_tensor("y", [EPC, CAP, D], f32, kind="ExternalOutput")

    relu = mybir.ActivationFunctionType.Relu

    with tile.TileContext(nc) as tc:
        with (
            tc.tile_pool(name="wts", bufs=2) as wts,
            tc.tile_pool(name="acts", bufs=2) as acts,
            tc.tile_pool(name="cst", bufs=1) as cst,
            tc.tile_pool(name="ps1", bufs=4, space="PSUM") as ps1,
            tc.tile_pool(name="ps2", bufs=2, space="PSUM") as ps2,
        ):
            # Single up-front DMAs for tokens + biases; a dummy ReLU reading
            # them advances the ACT engine past the DMA sem and pays the
            # activation-table load once, so steady-state Activations carry
            # only their PSUM wait (the ISA allows very few waits per ACT).
            xtt = cst.tile([128, EPC, DC, CAP], bf, name="xtt")
            b1t = cst.tile([128, EPC, HC], f32, name="b1t")
            nc.gpsimd.dma_start(xtt[:], xt[:])
            nc.gpsimd.dma_start(b1t[:], b1[:])
            scratch = cst.tile([128, 1], f32, name="scratch")
            nc.scalar.activation(scratch[:], b1t[:, 0, 0:1], relu,
                                 bias=b1t[:, 0, 0:1])

            for i in range(EPC):
                w1t = wts.tile([128, DC, H], bf, name="w1t")
                w2t = wts.tile([128, HC, D], bf, name="w2t")
                nc.gpsimd.dma_start(w1t[:], w1[i])
                nc.gpsimd.dma_start(w2t[:], w2[i])

                # h^T = relu(W1^T x^T + b1), produced [h, token] so the second
                # matmul can contract over h on the partition dim.
                ht = acts.tile([128, HC, CAP], bf, name="ht")
                for j in range(HC):
                    p1 = ps1.tile([128, CAP], f32, name="p1")
                    for c in range(DC):
                        nc.tensor.matmul(
                            p1[:],
                            w1t[:, c, j * 128:(j + 1) * 128],
                            xtt[:, i, c, :],
                            start=(c == 0),
                            stop=(c == DC - 1),
                        )
                    nc.scalar.activation(ht[:, j, :], p1[:], relu,
                                         bias=b1t[:, i, j:j + 1])

                # y = h @ W2  ->  [token, d]
                p2 = ps2.tile([128, D], f32, name="p2")
                for j in range(HC):
                    nc.tensor.matmul(p2[:], ht[:, j, :], w2t[:, j, :],
                                     start=(j == 0), stop=(j == HC - 1))
                yt = acts.tile([128, D], f32, name="yt")
                nc.vector.tensor_copy(yt[:], p2[:])
                nc.gpsimd.dma_start(y[i], yt[:])

    # Bacc lowering: splits excess per-instruction sem waits onto
    # InstEventSemaphore, moves matmul waits onto ldweights, inserts
    # activation table loads -- required for walrus codegen (1 wait slot
    # per 64B ISA instruction).
    nc.compile()
    return nc


def _get_nc():
    if "nc" not in _NC_CACHE:
        _NC_CACHE["nc"] = _build_bass()
    return _NC_CACHE["nc"]


def kernel(x, Wg, bg, W1, b1, W2, b2, gamma, beta):
    global LAST_RESULT
    x = np.asarray(x, np.float32)
    Wg = np.asarray(Wg, np.float32)
    bg = np.asarray(bg, np.float32)
    W1 = np.asarray(W1, np.float32)
    b1 = np.asarray(b1, np.float32)
    W2 = np.asarray(W2, np.float32)
    b2 = np.asarray(b2, np.float32)
    gamma = np.asarray(gamma, np.float32)
    beta = np.asarray(beta, np.float32)

    xf = x.reshape(T, D)

    # ---- gating: softmax over experts, top-2 (ties -> lower index, as top_k)
    logits = xf @ Wg + bg
    logits -= logits.max(-1, keepdims=True)
    probs = np.exp(logits)
    probs /= probs.sum(-1, keepdims=True)
    idx = np.argsort(-probs, axis=-1, kind="stable")[:, :TOPK]   # [T, K]
    vals = np.take_along_axis(probs, idx, axis=-1)               # [T, K]

    # ---- per-expert token lists (the all-to-all "sharding by routing")
    slot = np.full((T, TOPK), -1, np.int64)
    toks_per_e = []
    overflow = []  # (expert, token_ids) pairs beyond CAP -> host fallback
    for e in range(E):
        te = np.nonzero((idx == e).any(-1))[0]
        if len(te) > CAP:
            overflow.append((e, te[CAP:]))
            te = te[:CAP]
        toks_per_e.append(te)
        if len(te):
            k_of = (idx[te] == e).argmax(-1)
            slot[te, k_of] = np.arange(len(te))

    # ---- pack per-core device inputs (layouts match SBUF tiles exactly)
    xth = np.zeros((E, 128, DC, CAP), BF16)
    for e in range(E):
        te = toks_per_e[e]
        if len(te):
            blk = xf[te].T.reshape(DC, 128, len(te)).transpose(1, 0, 2)
            xth[e, :, :, :len(te)] = blk.astype(BF16)
    w1h = np.ascontiguousarray(
        W1.astype(BF16).reshape(E, DC, 128, H).transpose(0, 2, 1, 3))
    w2h = np.ascontiguousarray(
        W2.astype(BF16).reshape(E, HC, 128, D).transpose(0, 2, 1, 3))
    b1h = np.ascontiguousarray(b1.reshape(E, HC, 128).transpose(0, 2, 1))

    in_maps = []
    for c in range(NCORES):
        sl = slice(c * EPC, (c + 1) * EPC)
        in_maps.append({
            "xt": np.ascontiguousarray(xth[sl].transpose(1, 0, 2, 3)),
            "w1": w1h[sl],
            "w2": w2h[sl],
            "b1": np.ascontiguousarray(b1h[sl].transpose(1, 0, 2)),
        })

    # ---- run on the 8 cores
    from concourse.bass_utils import run_bass_kernel_spmd
    nc = _get_nc()
    res = run_bass_kernel_spmd(nc, in_maps, list(range(NCORES)),
                               trace=PROFILE)
    LAST_RESULT = res
    y_all = np.concatenate([r["y"] for r in res.results], axis=0)  # [E,CAP,D]

    # ---- unshard: scatter expert outputs back by routing, combine, LN
    ok = slot >= 0
    sl = np.where(ok, slot, 0)
    contrib = y_all[idx, sl] + b2[idx]                 # [T, K, D]
    out = xf + (vals[..., None] * contrib * ok[..., None]).sum(1)

    for e, te in overflow:  # practically never taken (CAP >> max count)
        k_of = (idx[te] == e).argmax(-1)
        w = vals[te, k_of]
        h = np.maximum(xf[te] @ W1[e] + b1[e], 0.0)
        out[te] += w[:, None] * (h @ W2[e] + b2[e])

    mu = out.mean(-1, keepdims=True)
    var = ((out - mu) ** 2).mean(-1, keepdims=True)
    o = (out - mu) / np.sqrt(var + EPS) * gamma + beta
    return o.reshape(B, S, D).astype(np.float32)


# revision 10
# speedup vs baseline: 1.1373x; 1.1373x over previous
"""Expert-parallel sparse MoE kernel for Trainium2 (8 NeuronCores).

Reference model: dense MoE (every expert on every token) followed by a
top-2-sparse combine, residual add, and LayerNorm.  Mathematically only the
top-2 experts per token contribute to the output, so the kernel routes each
token to its top-2 experts and only computes those expert FFNs.

Sharding: expert-parallel.  Each of the 8 cores owns 8 of the 64 experts and
receives the tokens routed to them (all-to-all by routing, done host-side as
part of sharding).  The device streams the expert weights (the dominant
memory traffic, cast to bf16) and computes y_e = relu(x @ W1[e] + b1[e]) @
W2[e] for every routed token.  The host applies the gate weights + b2 during
the unshard/scatter, adds the residual, and normalizes.
"""

import numpy as np
import ml_dtypes

B, S, D, H, E, TOPK = 2, 1024, 512, 2048, 64, 2
T = B * S
NCORES = 8
EPC = E // NCORES          # experts per core
CAP = 128                  # token capacity per expert (observed max ~95)
DC = D // 128              # 4 contraction chunks for x @ W1
HC = H // 128              # 16 contraction chunks for h @ W2
EPS = 1e-5
BF16 = ml_dtypes.bfloat16

PROFILE = False            # set True (module-level) to capture an NTFF trace
LAST_RESULT = None         # BassKernelResults of the last run (for test.py)

_NC_CACHE = {}


def _build_bass():
    """Build the per-core Bass/Tile program (identical on all 8 cores)."""
    import concourse.bacc as bacc
    import concourse.mybir as mybir
    from concourse import tile

    nc = bacc.Bacc("TRN2", target_bir_lowering=False, debug=False,
                   num_devices=NCORES)

    bf = mybir.dt.bfloat16
    f32 = mybir.dt.float32
    xt = nc.dram_tensor("xt", [128, EPC, DC, CAP], bf, kind="ExternalInput")
    # W1|W2 fused per expert: [d-part, DC*H (w1) + HC*D (w2)] -> one 4MB DMA
    w12 = nc.dram_tensor("w12", [EPC, 128, DC * H + HC * D], bf,
                         kind="ExternalInput")
    b1 = nc.dram_tensor("b1", [128, EPC, HC], f32, kind="ExternalInput")
    y = nc.dram_tensor("y", [EPC, CAP, D], f32, kind="ExternalOutput")

    relu = mybir.ActivationFunctionType.Relu

    with tile.TileContext(nc) as tc:
        with (
            tc.tile_pool(name="wts", bufs=2) as wts,
            tc.tile_pool(name="acts", bufs=2) as acts,
            tc.tile_pool(name="cst", bufs=1) as cst,
            tc.tile_pool(name="ps1", bufs=4, space="PSUM") as ps1,
            tc.tile_pool(name="ps2", bufs=2, space="PSUM") as ps2,
        ):
            # Single up-front DMAs for tokens + biases; a dummy ReLU reading
            # them advances the ACT engine past the DMA sem and pays the
            # activation-table load once, so steady-state Activations carry
            # only their PSUM wait (the ISA allows very few waits per ACT).
            xtt = cst.tile([128, EPC, DC, CAP], bf, name="xtt")
            b1t = cst.tile([128, EPC, HC], f32, name="b1t")
            nc.sync.dma_start(xtt[:], xt[:])
            nc.sync.dma_start(b1t[:], b1[:])
            scratch = cst.tile([128, 1], f32, name="scratch")
            nc.scalar.activation(scratch[:], b1t[:, 0, 0:1], relu,
                                 bias=b1t[:, 0, 0:1])

            for i in range(EPC):
                # Weight prefetch on the SWDGE (gpsimd) queue; everything else
                # on HWDGE so weight streaming never stalls behind compute.
                wt = wts.tile([128, DC * H + HC * D], bf, name="wt")
                nc.gpsimd.dma_start(wt[:], w12[i])
                w1t = wt[:, :DC * H].rearrange("p (c h) -> p c h", c=DC)
                w2t = wt[:, DC * H:].rearrange("p (c dd) -> p c dd", c=HC)

                # h^T = relu(W1^T x^T + b1), produced [h, token] so the second
                # matmul can contract over h on the partition dim.
                ht = acts.tile([128, HC, CAP], bf, name="ht")
                for j in range(HC):
                    p1 = ps1.tile([128, CAP], f32, name="p1")
                    for c in range(DC):
                        nc.tensor.matmul(
                            p1[:],
                            w1t[:, c, j * 128:(j + 1) * 128],
                            xtt[:, i, c, :],
                            start=(c == 0),
                            stop=(c == DC - 1),
                        )
                    nc.scalar.activation(ht[:, j, :], p1[:], relu,
                                         bias=b1t[:, i, j:j + 1])

                # y = h @ W2  ->  [token, d]
                p2 = ps2.tile([128, D], f32, name="p2")
                for j in range(HC):
                    nc.tensor.matmul(p2[:], ht[:, j, :], w2t[:, j, :],
                                     start=(j == 0), stop=(j == HC - 1))
                yt = acts.tile([128, D], f32, name="yt")
                nc.vector.tensor_copy(yt[:], p2[:])
                nc.sync.dma_start(y[i], yt[:])

    # Bacc lowering: splits excess per-instruction sem waits onto
    # InstEventSemaphore, moves matmul waits onto ldweights, inserts
    # activation table loads -- required for walrus codegen (1 wait slot
    # per 64B ISA instruction).
    nc.compile()
    return nc


def _get_nc():
    if "nc" not in _NC_CACHE:
        _NC_CACHE["nc"] = _build_bass()
    return _NC_CACHE["nc"]


def kernel(x, Wg, bg, W1, b1, W2, b2, gamma, beta):
    global LAST_RESULT
    x = np.asarray(x, np.float32)
    Wg = np.asarray(Wg, np.float32)
    bg = np.asarray(bg, np.float32)
    W1 = np.asarray(W1, np.float32)
    b1 = np.asarray(b1, np.float32)
    W2 = np.asarray(W2, np.float32)
    b2 = np.asarray(b2, np.float32)
    gamma = np.asarray(gamma, np.float32)
    beta = np.asarray(beta, np.float32)

    xf = x.reshape(T, D)

    # ---- gating: softmax over experts, top-2 (ties -> lower index, as top_k)
    logits = xf @ Wg + bg
    logits -= logits.max(-1, keepdims=True)
    probs = np.exp(logits)
    probs /= probs.sum(-1, keepdims=True)
    idx = np.argsort(-probs, axis=-1, kind="stable")[:, :TOPK]   # [T, K]
    vals = np.take_along_axis(probs, idx, axis=-1)               # [T, K]

    # ---- per-expert token lists (the all-to-all "sharding by routing")
    slot = np.full((T, TOPK), -1, np.int64)
    toks_per_e = []
    overflow = []  # (expert, token_ids) pairs beyond CAP -> host fallback
    for e in range(E):
        te = np.nonzero((idx == e).any(-1))[0]
        if len(te) > CAP:
            overflow.append((e, te[CAP:]))
            te = te[:CAP]
        toks_per_e.append(te)
        if len(te):
            k_of = (idx[te] == e).argmax(-1)
            slot[te, k_of] = np.arange(len(te))

    # ---- pack per-core device inputs (layouts match SBUF tiles exactly)
    xth = np.zeros((E, 128, DC, CAP), BF16)
    for e in range(E):
        te = toks_per_e[e]
        if len(te):
            blk = xf[te].T.reshape(DC, 128, len(te)).transpose(1, 0, 2)
            xth[e, :, :, :len(te)] = blk.astype(BF16)
    w1h = W1.astype(BF16).reshape(E, DC, 128, H).transpose(0, 2, 1, 3)
    w2h = W2.astype(BF16).reshape(E, HC, 128, D).transpose(0, 2, 1, 3)
    w12h = np.concatenate([w1h.reshape(E, 128, DC * H),
                           w2h.reshape(E, 128, HC * D)], axis=2)
    b1h = np.ascontiguousarray(b1.reshape(E, HC, 128).transpose(0, 2, 1))

    in_maps = []
    for c in range(NCORES):
        sl = slice(c * EPC, (c + 1) * EPC)
        in_maps.append({
            "xt": np.ascontiguousarray(xth[sl].transpose(1, 0, 2, 3)),
            "w12": w12h[sl],
            "b1": np.ascontiguousarray(b1h[sl].transpose(1, 0, 2)),
        })

    # ---- run on the 8 cores
    from concourse.bass_utils import run_bass_kernel_spmd
    nc = _get_nc()
    res = run_bass_kernel_spmd(nc, in_maps, list(range(NCORES)),
                               trace=PROFILE)
    LAST_RESULT = res
    y_all = np.concatenate([r["y"] for r in res.results], axis=0)  # [E,CAP,D]

    # ---- unshard: scatter expert outputs back by routing, combine, LN
    ok = slot >= 0
    sl = np.where(ok, slot, 0)
    contrib = y_all[idx, sl] + b2[idx]                 # [T, K, D]
    out = xf + (vals[..., None] * contrib * ok[..., None]).sum(1)

    for e, te in overflow:  # practically never taken (CAP >> max count)
        k_of = (idx[te] == e).argmax(-1)
        w = vals[te, k_of]
        h = np.maximum(xf[te] @ W1[e] + b1[e], 0.0)
        out[te] += w[:, None] * (h @ W2[e] + b2[e])

    mu = out.mean(-1, keepdims=True)
    var = ((out - mu) ** 2).mean(-1, keepdims=True)
    o = (out - mu) / np.sqrt(var + EPS) * gamma + beta
    return o.reshape(B, S, D).astype(np.float32)


# revision 12
# speedup vs baseline: 1.3493x; 1.1864x over previous
"""Expert-parallel sparse MoE kernel for Trainium2 (8 NeuronCores).

Reference model: dense MoE (every expert on every token) followed by a
top-2-sparse combine, residual add, and LayerNorm.  Mathematically only the
top-2 experts per token contribute to the output, so the kernel routes each
token to its top-2 experts and only computes those expert FFNs.

Sharding: expert-parallel.  Each of the 8 cores owns 8 of the 64 experts and
receives the tokens routed to them (all-to-all by routing, done host-side as
part of sharding).  The device streams the expert weights (the dominant
memory traffic, cast to bf16) and computes y_e = relu(x @ W1[e] + b1[e]) @
W2[e] for every routed token.  The host applies the gate weights + b2 during
the unshard/scatter, adds the residual, and normalizes.
"""

import numpy as np
import ml_dtypes

B, S, D, H, E, TOPK = 2, 1024, 512, 2048, 64, 2
T = B * S
NCORES = 8
EPC = E // NCORES          # experts per core
CAP = 128                  # token capacity per expert (observed max ~95)
DC = D // 128              # 4 contraction chunks for x @ W1
HC = H // 128              # 16 contraction chunks for h @ W2
EPS = 1e-5
BF16 = ml_dtypes.bfloat16

PROFILE = False            # set True (module-level) to capture an NTFF trace
LAST_RESULT = None         # BassKernelResults of the last run (for test.py)

_NC_CACHE = {}


def _build_bass():
    """Build the per-core Bass/Tile program (identical on all 8 cores)."""
    import concourse.bacc as bacc
    import concourse.mybir as mybir
    from concourse import tile

    nc = bacc.Bacc("TRN2", target_bir_lowering=False, debug=False,
                   num_devices=NCORES)

    bf = mybir.dt.bfloat16
    f32 = mybir.dt.float32
    xt = nc.dram_tensor("xt", [128, EPC, DC, CAP], bf, kind="ExternalInput")
    # W1|W2 fused per expert: [d-part, DC*H (w1) + HC*D (w2)] -> one 4MB DMA
    w12 = nc.dram_tensor("w12", [EPC, 128, DC * H + HC * D], bf,
                         kind="ExternalInput")
    b1 = nc.dram_tensor("b1", [128, EPC, HC], f32, kind="ExternalInput")
    y = nc.dram_tensor("y", [EPC, CAP, D], f32, kind="ExternalOutput")

    relu = mybir.ActivationFunctionType.Relu

    with tile.TileContext(nc) as tc:
        with (
            tc.tile_pool(name="wts", bufs=2) as wts,
            tc.tile_pool(name="acts", bufs=2) as acts,
            tc.tile_pool(name="cst", bufs=1) as cst,
            tc.tile_pool(name="ps1", bufs=4, space="PSUM") as ps1,
            tc.tile_pool(name="ps2", bufs=2, space="PSUM") as ps2,
        ):
            # Single up-front DMAs for tokens + biases; a dummy ReLU reading
            # them advances the ACT engine past the DMA sem and pays the
            # activation-table load once, so steady-state Activations carry
            # only their PSUM wait (the ISA allows very few waits per ACT).
            xtt = cst.tile([128, EPC, DC, CAP], bf, name="xtt")
            b1t = cst.tile([128, EPC, HC], f32, name="b1t")
            nc.sync.dma_start(xtt[:], xt[:])
            nc.sync.dma_start(b1t[:], b1[:])
            scratch = cst.tile([128, 1], f32, name="scratch")
            nc.scalar.activation(scratch[:], b1t[:, 0, 0:1], relu,
                                 bias=b1t[:, 0, 0:1])

            HH = HC // 2  # h-chunks per weight piece
            for i in range(EPC):
                # Weight prefetch on the SWDGE (gpsimd) queue in 4 x 1MB
                # pieces (separate tiles -> compute can start on the first
                # piece); everything else on HWDGE so weight streaming never
                # stalls behind compute.
                w1p = [wts.tile([128, DC, HH * 128], bf, name=f"w1p{h}")
                       for h in range(2)]
                w2p = [wts.tile([128, HH, D], bf, name=f"w2p{h}")
                       for h in range(2)]
                for h in range(2):
                    nc.gpsimd.dma_start(
                        w1p[h][:], w12[i][:, h * 4096:(h + 1) * 4096]
                        .rearrange("p (c hh) -> p c hh", c=DC))
                for h in range(2):
                    nc.gpsimd.dma_start(
                        w2p[h][:], w12[i][:, (2 + h) * 4096:(3 + h) * 4096]
                        .rearrange("p (c dd) -> p c dd", c=HH))

                # h^T = relu(W1^T x^T + b1), produced [h, token] so the second
                # matmul can contract over h on the partition dim.  mm2 for
                # chunk j-1 is interleaved after mm1 of chunk j to shorten the
                # per-expert critical path and keep PE dense.
                ht = acts.tile([128, HC, CAP], bf, name="ht")
                p2 = ps2.tile([128, D], f32, name="p2")
                for j in range(HC):
                    p1 = ps1.tile([128, CAP], f32, name="p1")
                    w1t = w1p[j // HH]
                    jj = j % HH
                    for c in range(DC):
                        nc.tensor.matmul(
                            p1[:],
                            w1t[:, c, jj * 128:(jj + 1) * 128],
                            xtt[:, i, c, :],
                            start=(c == 0),
                            stop=(c == DC - 1),
                        )
                    nc.scalar.activation(ht[:, j, :], p1[:], relu,
                                         bias=b1t[:, i, j:j + 1])
                    if j > 0:
                        nc.tensor.matmul(p2[:], ht[:, j - 1, :],
                                         w2p[(j - 1) // HH][:, (j - 1) % HH, :],
                                         start=(j - 1 == 0), stop=False,
                                         skip_group_check=True)
                nc.tensor.matmul(p2[:], ht[:, HC - 1, :],
                                 w2p[1][:, HH - 1, :],
                                 start=False, stop=True,
                                 skip_group_check=True)
                yt = acts.tile([128, D], f32, name="yt")
                nc.vector.tensor_copy(yt[:], p2[:])
                nc.sync.dma_start(y[i], yt[:])

    # Bacc lowering: splits excess per-instruction sem waits onto
    # InstEventSemaphore, moves matmul waits onto ldweights, inserts
    # activation table loads -- required for walrus codegen (1 wait slot
    # per 64B ISA instruction).
    nc.compile()
    return nc


def _get_nc():
    if "nc" not in _NC_CACHE:
        _NC_CACHE["nc"] = _build_bass()
    return _NC_CACHE["nc"]


def kernel(x, Wg, bg, W1, b1, W2, b2, gamma, beta):
    global LAST_RESULT
    x = np.asarray(x, np.float32)
    Wg = np.asarray(Wg, np.float32)
    bg = np.asarray(bg, np.float32)
    W1 = np.asarray(W1, np.float32)
    b1 = np.asarray(b1, np.float32)
    W2 = np.asarray(W2, np.float32)
    b2 = np.asarray(b2, np.float32)
    gamma = np.asarray(gamma, np.float32)
    beta = np.asarray(beta, np.float32)

    xf = x.reshape(T, D)

    # ---- gating: softmax over experts, top-2 (ties -> lower index, as top_k)
    logits = xf @ Wg + bg
    logits -= logits.max(-1, keepdims=True)
    probs = np.exp(logits)
    probs /= probs.sum(-1, keepdims=True)
    idx = np.argsort(-probs, axis=-1, kind="stable")[:, :TOPK]   # [T, K]
    vals = np.take_along_axis(probs, idx, axis=-1)               # [T, K]

    # ---- per-expert token lists (the all-to-all "sharding by routing")
    slot = np.full((T, TOPK), -1, np.int64)
    toks_per_e = []
    overflow = []  # (expert, token_ids) pairs beyond CAP -> host fallback
    for e in range(E):
        te = np.nonzero((idx == e).any(-1))[0]
        if len(te) > CAP:
            overflow.append((e, te[CAP:]))
            te = te[:CAP]
        toks_per_e.append(te)
        if len(te):
            k_of = (idx[te] == e).argmax(-1)
            slot[te, k_of] = np.arange(len(te))

    # ---- pack per-core device inputs (layouts match SBUF tiles exactly)
    xth = np.zeros((E, 128, DC, CAP), BF16)
    for e in range(E):
        te = toks_per_e[e]
        if len(te):
            blk = xf[te].T.reshape(DC, 128, len(te)).transpose(1, 0, 2)
            xth[e, :, :, :len(te)] = blk.astype(BF16)
    w1h = W1.astype(BF16).reshape(E, DC, 128, H).transpose(0, 2, 1, 3)
    w2h = W2.astype(BF16).reshape(E, HC, 128, D).transpose(0, 2, 1, 3)
    HH = HC // 2
    w12h = np.concatenate([
        w1h[:, :, :, :HH * 128].reshape(E, 128, DC * HH * 128),
        w1h[:, :, :, HH * 128:].reshape(E, 128, DC * HH * 128),
        w2h[:, :, :HH, :].reshape(E, 128, HH * D),
        w2h[:, :, HH:, :].reshape(E, 128, HH * D),
    ], axis=2)
    b1h = np.ascontiguousarray(b1.reshape(E, HC, 128).transpose(0, 2, 1))

    in_maps = []
    for c in range(NCORES):
        sl = slice(c * EPC, (c + 1) * EPC)
        in_maps.append({
            "xt": np.ascontiguousarray(xth[sl].transpose(1, 0, 2, 3)),
            "w12": w12h[sl],
            "b1": np.ascontiguousarray(b1h[sl].transpose(1, 0, 2)),
        })

    # ---- run on the 8 cores
    from concourse.bass_utils import run_bass_kernel_spmd
    nc = _get_nc()
    res = run_bass_kernel_spmd(nc, in_maps, list(range(NCORES)),
                               trace=PROFILE)
    LAST_RESULT = res
    y_all = np.concatenate([r["y"] for r in res.results], axis=0)  # [E,CAP,D]

    # ---- unshard: scatter expert outputs back by routing, combine, LN
    ok = slot >= 0
    sl = np.where(ok, slot, 0)
    contrib = y_all[idx, sl] + b2[idx]                 # [T, K, D]
    out = xf + (vals[..., None] * contrib * ok[..., None]).sum(1)

    for e, te in overflow:  # practically never taken (CAP >> max count)
        k_of = (idx[te] == e).argmax(-1)
        w = vals[te, k_of]
        h = np.maximum(xf[te] @ W1[e] + b1[e], 0.0)
        out[te] += w[:, None] * (h @ W2[e] + b2[e])

    mu = out.mean(-1, keepdims=True)
    var = ((out - mu) ** 2).mean(-1, keepdims=True)
    o = (out - mu) / np.sqrt(var + EPS) * gamma + beta
    return o.reshape(B, S, D).astype(np.float32)
